# revision 12
# baseline (speedup 1.0000x reference)
"""AttentionDecoderCell on 8 TRN2 NeuronCores.

Sharding: data-parallel over batch B=64 (8 rows per core) for the
Bahdanau-attention + GRU-state part; tensor-parallel (column shard,
4000 cols per core) for the three 32k-vocab output projections.
Cross-core traffic: AllGather of the attention context [64,256] and an
AllReduce of the vocab softmax denominators [64].

Device-side structure (per core):
  - x arrives as bf16; each batch is DMAed to SBUF in natural layout
    [128p,16t,256e] (for the context matmuls) and transposed from DRAM
    via one xbar DMA-transpose into [128e',2ch,2048r] so the
    TensorEngine can contract over e.
  - pre = x@Wa2 + stm@Wa1 accumulates in PSUM ([128 rows, 512c]); the
    stm part rides as a K=8 one-hot matmul so no bias pass is needed.
  - tanh on ScalarE (PSUM->SBUF bf16), then one fused VectorE
    tensor_tensor_reduce does (tanh * V_a) with add-accumulate -> et.
  - softmax over T skips max-subtraction (|et| <~ 4) and folds 1/sum
    into the context: context_raw = sum exp(et)*x via TensorE (exp
    values as the K=8-wide stationary operand), scaled once at the end.
  - vocab logits: lhsT = [inputsT; stmT; contextT] (features on
    partitions), rhs = [W_o;U_o;C_o] column shard, exp on ScalarE with
    fused accum for the softmax denominators.
"""

import numpy as np
import ml_dtypes

B, T, E, U, D, O = 64, 2048, 256, 256, 256, 32000
NC_ = 8
B_LOC = B // NC_          # 8
O_LOC = O // NC_          # 4000
NTB = T // 128            # 16 row-tiles per batch
OT = 500                  # vocab column tile
NOT = O_LOC // OT         # 8

_CACHE = {}
LAST_RESULT = None


def _build():
    if "nc" in _CACHE:
        return _CACHE["nc"]

    import concourse.mybir as mybir
    import concourse.tile as tile
    from concourse import bacc
    import concourse.bass as bass

    F32, BF16 = mybir.dt.float32, mybir.dt.bfloat16
    AF = mybir.ActivationFunctionType
    OP = mybir.AluOpType

    nc = bacc.Bacc("TRN2", target_bir_lowering=False, debug=False, num_devices=NC_)

    def inp(name, shape, dt=BF16):
        return nc.dram_tensor(name, shape, dt, kind="ExternalInput")

    x_d = inp("x", [B_LOC, T, E])                 # bf16
    xt_d = inp("xT", [B_LOC, 2, 128, T])          # bf16, host-transposed
    wa1_d = inp("wa1", [256, 512])
    wa2_d = inp("wa2", [256, 512])
    va_d = inp("va", [512])
    oh_d = inp("onehot", [8, 1024])
    ones_d = inp("ones", [128], F32)
    id_d = inp("ident", [128, 128], F32)
    wcat_d = inp("wcat", [768, O_LOC])
    bo_d = inp("bo", [O_LOC])
    wrz_d = inp("wrz", [768, 512])
    brz_d = inp("brz", [512])
    wp3_d = inp("wp3", [768, 256])
    bp_d = inp("bp", [256])
    inT_d = inp("inputsT", [256, 64])
    stT_d = inp("stmT", [256, 64])
    inTl_d = inp("inputsT_loc", [256, 8])
    stTl_d = inp("stmT_loc", [256, 8])
    stml_d = inp("stm_loc", [8, 256], F32)

    probs_d = nc.dram_tensor("probs", [B, O_LOC], F32, kind="ExternalOutput")
    st_d = nc.dram_tensor("st_out", [B_LOC, U], F32, kind="ExternalOutput")

    ctxl_d = nc.dram_tensor("ctx_loc", [B_LOC, E], F32)
    ctxa_d = nc.dram_tensor("ctx_all", [B, E], F32, addr_space="Shared")
    vsl_d = nc.dram_tensor("vs_loc", [B], F32)
    vsg_d = nc.dram_tensor("vs_glob", [B], F32, addr_space="Shared")
    RG = [list(range(NC_))]

    def col(ap_):  # [n] dram -> [n,1]
        return ap_.ap().rearrange("(p o) -> p o", o=1)

    def row(ap_):  # [n] dram -> [1,n]
        return ap_.ap().rearrange("(o f) -> o f", o=1)

    with tile.TileContext(nc) as tc:
        with (
            tc.tile_pool(name="singles", bufs=1) as S,
            tc.tile_pool(name="xnat", bufs=4) as XN,
            tc.tile_pool(name="xt", bufs=3) as XT,
            tc.tile_pool(name="tanh", bufs=3) as TH,
            tc.tile_pool(name="dump", bufs=2) as DU,
            tc.tile_pool(name="psA", bufs=2, space="PSUM") as PSA,
            tc.tile_pool(name="psCtx", bufs=1, space="PSUM") as PSC,
            tc.tile_pool(name="psS", bufs=3, space="PSUM") as PSS,
        ):
            # ---------- constants / small inputs (all plain HWDGE DMAs) ----------
            wa1_sb = S.tile([128, 2, 512], BF16)
            nc.sync.dma_start(out=wa1_sb, in_=wa1_d.ap().rearrange("(k p) c -> p k c", p=128))
            wa2_sb = S.tile([128, 2, 512], BF16)
            nc.sync.dma_start(out=wa2_sb, in_=wa2_d.ap().rearrange("(k p) c -> p k c", p=128))

            va_row = S.tile([1, 512], BF16)
            nc.sync.dma_start(out=va_row, in_=row(va_d))

            oh_sb = S.tile([8, 8, 128], BF16)
            nc.sync.dma_start(out=oh_sb, in_=oh_d.ap().rearrange("j (b m) -> j b m", b=8))

            ones_col_f = S.tile([128, 1], F32)
            nc.sync.dma_start(out=ones_col_f, in_=col(ones_d))
            ones_row_f = S.tile([1, 128], F32)
            nc.sync.dma_start(out=ones_row_f, in_=row(ones_d))
            ones_row_bf = S.tile([1, 128], BF16)
            nc.vector.tensor_copy(ones_row_bf, ones_row_f)
            id_sb = S.tile([128, 128], F32)
            nc.sync.dma_start(out=id_sb, in_=id_d.ap())

            # V_a replicated across partitions via a K=1 ones-matmul
            # (two copies side by side to match the [128,1024] tanh tiles)
            va_ps = PSA.tile([128, 1024], F32, tag="pre2")
            nc.tensor.matmul(va_ps[:, :512], ones_row_bf, va_row, start=True, stop=True)
            nc.tensor.matmul(va_ps[:, 512:], ones_row_bf, va_row, start=True, stop=True)
            va_rep = S.tile([128, 1024], BF16)
            nc.vector.tensor_copy(va_rep, va_ps)

            inT_sb = S.tile([128, 2, 64], BF16)
            nc.sync.dma_start(out=inT_sb, in_=inT_d.ap().rearrange("(k p) b -> p k b", p=128))
            stT_sb = S.tile([128, 2, 64], BF16)
            nc.sync.dma_start(out=stT_sb, in_=stT_d.ap().rearrange("(k p) b -> p k b", p=128))
            inTl_sb = S.tile([128, 2, 8], BF16)
            nc.sync.dma_start(out=inTl_sb, in_=inTl_d.ap().rearrange("(k p) b -> p k b", p=128))
            stTl_sb = S.tile([128, 2, 8], BF16)
            nc.sync.dma_start(out=stTl_sb, in_=stTl_d.ap().rearrange("(k p) b -> p k b", p=128))
            stml_sb = S.tile([8, 256], F32)
            nc.sync.dma_start(out=stml_sb, in_=stml_d.ap())

            wrz_sb = S.tile([128, 6, 512], BF16)
            nc.sync.dma_start(out=wrz_sb, in_=wrz_d.ap().rearrange("(k p) c -> p k c", p=128))
            brz_sb = S.tile([1, 512], BF16)
            nc.sync.dma_start(out=brz_sb, in_=row(brz_d))
            wp3_sb = S.tile([128, 6, 256], BF16)
            nc.sync.dma_start(out=wp3_sb, in_=wp3_d.ap().rearrange("(k p) c -> p k c", p=128))
            bp_sb = S.tile([1, 256], BF16)
            nc.sync.dma_start(out=bp_sb, in_=row(bp_d))
            bo_sb = S.tile([1, O_LOC], BF16)
            nc.sync.dma_start(out=bo_sb, in_=row(bo_d))

            # persistent attention state
            et_all = S.tile([128, 128], F32)
            sp_all = S.tile([128, 8], F32)
            A_sb = S.tile([128, 128, 8], BF16)
            nc.vector.memset(A_sb, 0.0)
            ctx_ps = PSC.tile([8, 256], F32)

            # pre1 = stm_loc @ Wa1  -> [8, 512]
            pre1_ps = PSS.tile([8, 512], F32, tag="sm")
            nc.tensor.matmul(pre1_ps, stTl_sb[:, 0, :], wa1_sb[:, 0, :], start=True, stop=False)
            nc.tensor.matmul(pre1_ps, stTl_sb[:, 1, :], wa1_sb[:, 1, :], start=False, stop=True)
            pre1_sb = S.tile([8, 512], BF16)
            nc.vector.tensor_copy(pre1_sb, pre1_ps)

            # ---------- main attention loop ----------
            def emit_mains(b):
                x_nat = XN.tile([128, NTB, 256], BF16, tag="xnat")
                nc.sync.dma_start(out=x_nat, in_=x_d.ap()[b].rearrange("(t p) e -> p t e", p=128))
                xT = XT.tile([128, 2, T], BF16, tag="xt")
                nc.sync.dma_start(out=xT, in_=xt_d.ap()[b].rearrange("ch p r -> p ch r"))
                for tp in range(NTB // 2):
                    ps = PSA.tile([128, 1024], F32, tag="pre2")
                    for h in (0, 1):
                        t = tp * 2 + h
                        rs = slice(t * 128, (t + 1) * 128)
                        half = ps[:, h * 512:(h + 1) * 512]
                        nc.tensor.matmul(half, xT[:, 0, rs], wa2_sb[:, 0, :], start=True, stop=False)
                        nc.tensor.matmul(half, xT[:, 1, rs], wa2_sb[:, 1, :], start=False, stop=False)
                        nc.tensor.matmul(half, oh_sb[:, b, :], pre1_sb, start=False, stop=True)
                    th = TH.tile([128, 1024], BF16, tag="tanh")
                    nc.scalar.activation(th, ps, AF.Tanh)
                    dmp = DU.tile([128, 1024], BF16, tag="dump")
                    nc.vector.tensor_tensor(dmp, th, va_rep, OP.mult)
                    for h in (0, 1):
                        gt = b * NTB + tp * 2 + h
                        nc.vector.tensor_reduce(
                            et_all[:, gt:gt + 1], dmp[:, h * 512:(h + 1) * 512],
                            axis=mybir.AxisListType.X, op=OP.add)
                return x_nat

            def emit_softmax_ctx(b, x_nat):
                sl = slice(NTB * b, NTB * (b + 1))
                nc.scalar.activation(A_sb[:, sl, b:b + 1], et_all[:, sl, None], AF.Exp)
                nc.vector.tensor_reduce(sp_all[:, b:b + 1], A_sb[:, sl, b:b + 1],
                                        axis=mybir.AxisListType.XY, op=OP.add)
                for j in range(NTB):
                    nc.tensor.matmul(ctx_ps, A_sb[:, NTB * b + j, :], x_nat[:, j, :],
                                     start=(b == 0 and j == 0),
                                     stop=(b == B_LOC - 1 and j == NTB - 1))

            x_nats = {}
            for b in range(B_LOC):
                x_nats[b] = emit_mains(b)
                if b >= 1:
                    emit_softmax_ctx(b - 1, x_nats[b - 1])
            emit_softmax_ctx(B_LOC - 1, x_nats[B_LOC - 1])

            # context normalization: ctx = ctx_raw / sum(exp)
            s8_ps = PSS.tile([8, 1], F32, tag="sm")
            nc.tensor.matmul(s8_ps, sp_all, ones_col_f, start=True, stop=True)
            r8 = S.tile([8, 1], F32)
            nc.vector.reciprocal(r8, s8_ps)
            ctx_sb = S.tile([8, 256], F32)
            nc.vector.tensor_scalar(out=ctx_sb, in0=ctx_ps, scalar1=r8,
                                    scalar2=None, op0=OP.mult)

            # vocab weights (emitted late so x DMAs win the queues early)
            wcat_sb = S.tile([128, 6, O_LOC], BF16)
            wcat_ap = wcat_d.ap().rearrange("(k p) c -> p k c", p=128)
            for k in range(6):
                nc.sync.dma_start(out=wcat_sb[:, k, :], in_=wcat_ap[:, k, :])

            # ---------- gather context across cores ----------
            nc.sync.dma_start(out=ctxl_d.ap(), in_=ctx_sb)
            nc.gpsimd.collective_compute(
                "AllGather", OP.bypass, replica_groups=RG,
                ins=[ctxl_d.ap()], outs=[ctxa_d.ap()])
            ctxall_sb = S.tile([64, 256], F32)
            nc.sync.dma_start(out=ctxall_sb, in_=ctxa_d.ap())

            ctxT = S.tile([128, 2, 64], BF16)
            for ch in (0, 1):
                tps = PSS.tile([128, 64], F32, tag="sm")
                nc.tensor.transpose(tps, ctxall_sb[:, ch * 128:(ch + 1) * 128], id_sb[:64, :64])
                nc.vector.tensor_copy(ctxT[:, ch, :], tps)

            # ---------- st (GRU state, local batches) ----------
            ctxTl = S.tile([128, 2, 8], BF16)
            for ch in (0, 1):
                tps = PSS.tile([128, 8], F32, tag="sm")
                nc.tensor.transpose(tps, ctx_sb[:, ch * 128:(ch + 1) * 128], id_sb[:8, :8])
                nc.vector.tensor_copy(ctxTl[:, ch, :], tps)

            gate_lhs = [inTl_sb[:, 0, :], inTl_sb[:, 1, :], stTl_sb[:, 0, :],
                        stTl_sb[:, 1, :], ctxTl[:, 0, :], ctxTl[:, 1, :]]
            g_ps = PSS.tile([8, 512], F32, tag="sm")
            for k in range(6):
                nc.tensor.matmul(g_ps, gate_lhs[k], wrz_sb[:, k, :], start=(k == 0), stop=False)
            nc.tensor.matmul(g_ps, ones_row_bf[:, :8], brz_sb, start=False, stop=True)
            eneg = S.tile([8, 512], F32)
            nc.scalar.activation(eneg, g_ps, AF.Exp, scale=-1.0)
            t1 = S.tile([8, 512], F32)
            nc.vector.tensor_scalar_add(t1, eneg, 1.0)
            sig = S.tile([8, 512], F32)
            nc.vector.reciprocal(sig, t1)
            rtstm = S.tile([8, 256], F32)
            nc.vector.tensor_mul(rtstm, sig[:, :256], stml_sb)
            rtT = S.tile([128, 2, 8], BF16)
            for ch in (0, 1):
                tps = PSS.tile([128, 8], F32, tag="sm")
                nc.tensor.transpose(tps, rtstm[:, ch * 128:(ch + 1) * 128], id_sb[:8, :8])
                nc.vector.tensor_copy(rtT[:, ch, :], tps)
            p_lhs = [inTl_sb[:, 0, :], inTl_sb[:, 1, :], rtT[:, 0, :],
                     rtT[:, 1, :], ctxTl[:, 0, :], ctxTl[:, 1, :]]
            p_ps = PSS.tile([8, 256], F32, tag="sm")
            for k in range(6):
                nc.tensor.matmul(p_ps, p_lhs[k], wp3_sb[:, k, :], start=(k == 0), stop=False)
            nc.tensor.matmul(p_ps, ones_row_bf[:, :8], bp_sb, start=False, stop=True)
            stp = S.tile([8, 256], F32)
            nc.scalar.activation(stp, p_ps, AF.Tanh)
            dlt = S.tile([8, 256], F32)
            nc.vector.tensor_tensor(dlt, stp, stml_sb, OP.subtract)
            zd = S.tile([8, 256], F32)
            nc.vector.tensor_tensor(zd, sig[:, 256:], dlt, OP.mult)
            st_sb = S.tile([8, 256], F32)
            nc.vector.tensor_tensor(st_sb, stml_sb, zd, OP.add)
            nc.sync.dma_start(out=st_d.ap(), in_=st_sb)

            # ---------- vocab projections + softmax ----------
            vocab_lhs = [inT_sb[:, 0, :], inT_sb[:, 1, :], stT_sb[:, 0, :],
                         stT_sb[:, 1, :], ctxT[:, 0, :], ctxT[:, 1, :]]
            probs_sb = S.tile([64, O_LOC], F32)
            vs_all = S.tile([64, NOT], F32)
            for n in range(NOT):
                cs = slice(n * OT, (n + 1) * OT)
                vps = PSA.tile([64, OT], F32, tag="pre2")
                for k in range(6):
                    nc.tensor.matmul(vps, vocab_lhs[k], wcat_sb[:, k, cs], start=(k == 0), stop=False)
                nc.tensor.matmul(vps, ones_row_bf[:, :64], bo_sb[:, cs], start=False, stop=True)
                nc.scalar.activation(probs_sb[:, cs], vps, AF.Exp)
                nc.vector.tensor_reduce(vs_all[:, n:n + 1], probs_sb[:, cs],
                                        axis=mybir.AxisListType.X, op=OP.add)
            vs_loc = S.tile([64, 1], F32)
            nc.vector.tensor_reduce(vs_loc, vs_all, axis=mybir.AxisListType.X, op=OP.add)
            nc.sync.dma_start(out=col(vsl_d), in_=vs_loc)
            nc.gpsimd.collective_compute(
                "AllReduce", OP.add, replica_groups=RG,
                ins=[vsl_d.ap()], outs=[vsg_d.ap()])
            vg_sb = S.tile([64, 1], F32)
            nc.sync.dma_start(out=vg_sb, in_=col(vsg_d))
            rv = S.tile([64, 1], F32)
            nc.vector.reciprocal(rv, vg_sb)
            nc.vector.tensor_scalar(out=probs_sb, in0=probs_sb, scalar1=rv,
                                    scalar2=None, op0=OP.mult)
            nc.sync.dma_start(out=probs_d.ap(), in_=probs_sb)

    nc.compile()
    _CACHE["nc"] = nc
    return nc


def _prep(inputs):
    f = np.float32
    bf = ml_dtypes.bfloat16

    def cb(a):
        return np.ascontiguousarray(np.asarray(a, dtype=f).astype(bf))

    def c(a):
        return np.ascontiguousarray(a, dtype=f)

    x_seq = inputs["x_seq"]
    inp_, stm = inputs["inputs"], inputs["stm"]
    W_a, V_a = inputs["W_a"], inputs["V_a"]

    shared = {
        "wa1": cb(W_a[:U]),
        "wa2": cb(W_a[U:]),
        "va": cb(V_a),
        "ones": np.ones(128, f),
        "ident": np.eye(128, dtype=f),
        "wrz": cb(np.concatenate([
            np.concatenate([inputs["W_r"], inputs["W_z"]], 1),
            np.concatenate([inputs["U_r"], inputs["U_z"]], 1),
            np.concatenate([inputs["C_r"], inputs["C_z"]], 1)], 0)),
        "brz": cb(np.concatenate([inputs["b_r"], inputs["b_z"]])),
        "wp3": cb(np.concatenate([inputs["W_p"], inputs["U_p"], inputs["C_p"]], 0)),
        "bp": cb(inputs["b_p"]),
        "inputsT": cb(np.asarray(inp_).T),
        "stmT": cb(np.asarray(stm).T),
    }
    oh = np.zeros((8, 1024), f)
    for b in range(8):
        oh[b, b * 128:(b + 1) * 128] = 1.0
    shared["onehot"] = cb(oh)

    x_bf = np.asarray(x_seq, dtype=f).astype(bf)
    inT = np.asarray(inp_, dtype=f).T
    stT = np.asarray(stm, dtype=f).T

    in_maps = []
    for i in range(NC_):
        sb = slice(i * B_LOC, (i + 1) * B_LOC)
        so = slice(i * O_LOC, (i + 1) * O_LOC)
        m = dict(shared)
        m["x"] = np.ascontiguousarray(x_bf[sb])
        m["xT"] = np.ascontiguousarray(
            x_bf[sb].transpose(0, 2, 1).reshape(B_LOC, 2, 128, T))
        m["wcat"] = cb(np.concatenate(
            [inputs["W_o"][:, so], inputs["U_o"][:, so], inputs["C_o"][:, so]], 0))
        m["bo"] = cb(inputs["b_o"][so])
        m["inputsT_loc"] = cb(inT[:, sb])
        m["stmT_loc"] = cb(stT[:, sb])
        m["stm_loc"] = c(stm[sb])
        in_maps.append(m)
    return in_maps


def kernel(**inputs):
    global LAST_RESULT
    from concourse.bass_utils import run_bass_kernel_spmd

    nc = _build()
    in_maps = _prep(inputs)
    res = run_bass_kernel_spmd(nc, in_maps, core_ids=list(range(NC_)))
    LAST_RESULT = res
    probs = np.concatenate([res.results[i]["probs"] for i in range(NC_)], axis=1)
    st = np.concatenate([res.results[i]["st_out"] for i in range(NC_)], axis=0)
    return probs.astype(np.float32), st.astype(np.float32)
